# revision 48
# baseline (speedup 1.0000x reference)
"""Trainium2 Bass kernel for a dense transformer block (causal MHA + FFN, post-LN).

Sharding over 8 NeuronCores:
  - Attention is tensor-parallel over heads: core c computes heads 2c, 2c+1
    for all 4096 tokens (B*T flattened, batch-major).
  - One AllToAll per batch redistributes the per-head attention outputs so
    core c ends up with the full head-concatenated attention output
    (transposed) for its token half-slices: batch-0 tokens [256c, 256c+256)
    and batch-1 tokens [256c, 256c+256).
  - Wo + residual + LN1 + FFN + residual + LN2 are sequence-parallel: each
    core processes its 2x256 token rows and outputs [512, 1024].

Matmuls run in bf16 (fp32 PSUM accumulation), except ffn1 which runs in fp8
e4m3 DoubleRowSwInterleave (2 contraction rows per cycle; W1 and x1T stored
as fp8). Softmax runs without the max subtraction (score range is +-2 here,
exp cannot overflow), with the softmax denominator obtained for free as an
extra ones-column in the P@V matmul. Residuals / layernorms are fp32; the
LN1 apply is bf16.

Overlap structure: batch-1 qkv chunks are interleaved into batch-0's
attention i-blocks; the back-half constants stream after the a2a0 dispatch;
wo/LN1 for batch-0 rows starts as soon as attn1 drains (its gather is issued
on two DMA rings), and the first 20 fo's of ffn1 run on the finished token
half to bridge the a2a1 + gather latency before wo/LN1 of batch-1 rows; ffn2
runs in two m-group passes so the LN2/store epilogue overlaps pass-1 matmuls.
"""

import sys

sys.path.insert(0, "/opt/trn_rl_repo")

import numpy as np
import ml_dtypes

B, T, E, H = 2, 2048, 1024, 16
HS = E // H  # 64
N_CORES = 8
HPC = H // N_CORES  # heads per core = 2
NTOK = B * T  # 4096
TSL = NTOK // N_CORES  # 512 token rows per core
HSL = TSL // B  # 256 rows per (core, batch) half-slice
EPS = 1e-5

BF16 = ml_dtypes.bfloat16
F8E4 = ml_dtypes.float8_e4m3fn
EO_ = E // 128

_cache = {}


def _build(n_cores=N_CORES):
    import concourse.bass as bass
    import concourse.tile as tile
    import concourse.bacc as bacc
    from concourse import mybir

    BF = mybir.dt.bfloat16
    F32 = mybir.dt.float32
    F8 = mybir.dt.float8e4
    DR = mybir.MatmulPerfMode.DoubleRow
    DRSI = mybir.MatmulPerfMode.DoubleRowSwInterleave
    AF = mybir.ActivationFunctionType
    OP = mybir.AluOpType

    nc = bacc.Bacc("TRN2", target_bir_lowering=False, debug=False,
                   num_devices=n_cores)

    EO = E // 128            # 8 chunks of the embedding dim
    FO = 4 * E // 128        # 32 chunks of the FFN hidden dim
    TC = T // 512            # 4 t-chunks of 512 per batch

    # ---- I/O (host passes pre-tiled layouts: 1 contiguous run/partition) --
    xT_d = nc.dram_tensor("xT", [128, NTOK // 512, EO, 512], BF,
                          kind="ExternalInput")
    xsl_d = nc.dram_tensor("x_slice", [128, TSL // 128, E], F32,
                           kind="ExternalInput")
    wq_d = nc.dram_tensor("wq", [128, E // 128, HPC * HS], BF,
                          kind="ExternalInput")
    wk_d = nc.dram_tensor("wk", [128, E // 128, HPC * HS], BF,
                          kind="ExternalInput")
    wv_d = nc.dram_tensor("wv", [128, E // 128, HPC * HS], BF,
                          kind="ExternalInput")
    wo_d = nc.dram_tensor("wo", [128, EO, E], BF, kind="ExternalInput")
    # W1 pre-interleaved for DoubleRowSwInterleave: per (fo, eo-pair) the 256
    # columns are [A127, B127, A126, ..., B0] (pair-interleaved, reversed)
    w1_d = nc.dram_tensor("w1", [128, 4 * E // 128, EO // 2, 256], F8,
                          kind="ExternalInput")
    w2_d = nc.dram_tensor("w2", [4 * E, E], BF, kind="ExternalInput")
    b1s_d = nc.dram_tensor("b1s", [128, 4 * E // 128], F32,
                           kind="ExternalInput")
    bo_d = nc.dram_tensor("bo", [E], F32, kind="ExternalInput")
    b1_d = nc.dram_tensor("b1", [4 * E], F32, kind="ExternalInput")
    b2_d = nc.dram_tensor("b2", [E], F32, kind="ExternalInput")
    g1_d = nc.dram_tensor("g1", [E], F32, kind="ExternalInput")
    be1_d = nc.dram_tensor("be1", [E], F32, kind="ExternalInput")
    g2_d = nc.dram_tensor("g2", [E], F32, kind="ExternalInput")
    be2_d = nc.dram_tensor("be2", [E], F32, kind="ExternalInput")
    masks_d = nc.dram_tensor("masks", [128, 4, 512], BF, kind="ExternalInput")
    idb_d = nc.dram_tensor("id_bf", [128, 128], BF, kind="ExternalInput")
    out_d = nc.dram_tensor("out", [TSL, E], F32, kind="ExternalOutput")

    def bcast_ap(d, n):
        # [n]-vector in DRAM broadcast across 128 partitions
        a = d.ap()
        return bass.AP(tensor=a.tensor, offset=a.offset, ap=[[0, 128], [1, n]])

    with tile.TileContext(nc) as tc:
        with tc.tile_pool(name="dram", bufs=1, space="DRAM") as dram, \
             tc.tile_pool(name="consts", bufs=1) as consts:

            a2a_in = [dram.tile([n_cores, 128, HSL], BF, name=f"a2a_in{b}")
                      for b in range(B)]
            a2a_out = [dram.tile([n_cores, 128, HSL], BF, name=f"a2a_out{b}")
                       for b in range(B)]

            # ---- attention-critical constants first ---------------------
            wq_sb = consts.tile([128, EO, HPC * HS], BF)
            nc.scalar.dma_start(wq_sb[:], wq_d.ap())
            wk_sb = consts.tile([128, EO, HPC * HS], BF)
            wv_sb = consts.tile([128, EO, HPC * HS], BF)
            masks_sb = consts.tile([128, 4, 512], BF)
            idb_sb = consts.tile([128, 128], BF)
            eps_sb = consts.tile([128, 1], F32)
            nc.vector.memset(eps_sb[:], EPS)
            # back-half constants: tiles allocated here, but their DMAs are
            # deferred until after the a2a0 dispatch so the startup DMA
            # engines are fully available for the xT stream
            xsl_sb = consts.tile([128, TSL // 128, E], F32)
            wo_sb = consts.tile([128, EO, E], BF)
            b1_sb = consts.tile([128, FO], F32)
            bo_bc = consts.tile([128, E], F32)
            b2_bc = consts.tile([128, E], F32)
            g1_bc = consts.tile([128, E], F32)
            be1_bc = consts.tile([128, E], F32)
            g2_bc = consts.tile([128, E], F32)
            be2_bc = consts.tile([128, E], F32)

            def load_back_half_consts():
                nc.gpsimd.dma_start(xsl_sb[:], xsl_d.ap())
                nc.gpsimd.dma_start(wo_sb[:], wo_d.ap())
                nc.gpsimd.dma_start(b1_sb[:], b1s_d.ap())
                nc.gpsimd.dma_start(bo_bc[:], bcast_ap(bo_d, E))
                nc.gpsimd.dma_start(b2_bc[:], bcast_ap(b2_d, E))
                nc.gpsimd.dma_start(g1_bc[:], bcast_ap(g1_d, E))
                nc.gpsimd.dma_start(be1_bc[:], bcast_ap(be1_d, E))
                nc.gpsimd.dma_start(g2_bc[:], bcast_ap(g2_d, E))
                nc.gpsimd.dma_start(be2_bc[:], bcast_ap(be2_d, E))
            # x + bo precomputed once; saves one DVE add per Wo psum tile
            # (computed after the attention loop so it can't stall qkv copies)
            xpb_sb = consts.tile([128, TSL // 128, E], F32)

            # persistent home for the post-a2a gathered attention output so
            # its DMA can be issued right after each collective
            hcT = [consts.tile([128, EO, HSL], BF, tag=f"hcT{b}",
                               name=f"hcT{b}")
                   for b in range(B)]

            # ================= attention (heads 2c, 2c+1) =================
            with tc.tile_pool(name="att_big", bufs=1) as att_big, \
                 tc.tile_pool(name="att_qkv", bufs=2) as att_qkv, \
                 tc.tile_pool(name="att_pt", bufs=4) as att_pt, \
                 tc.tile_pool(name="att_small", bufs=4) as att_small, \
                 tc.tile_pool(name="ps_big", bufs=2, space="PSUM") as ps_big, \
                 tc.tile_pool(name="ps_small", bufs=2, space="PSUM") as ps_small, \
                 tc.tile_pool(name="ps_av", bufs=2, space="PSUM") as ps_av:
                ps_qk = ps_s = ps_big          # share 2x 2-bank slots (tag "qs")
                ps_v = ps_tp = ps_small        # share 2x 1-bank slots (tag "vtp")

                # [p, t-chunk, eo, 512] — slice-major so one DMA per t-chunk
                # is a single contiguous 8KB run per partition; chunks split
                # across the sync + scalar HWDGE rings so qkv0 never starves
                xT_sb = att_big.tile([128, NTOK // 512, EO, 512], BF, tag="xT")
                nc.sync.dma_start(xT_sb[:, 0], xT_d.ap()[:, 0])
                nc.scalar.dma_start(wk_sb[:], wk_d.ap())
                nc.scalar.dma_start(wv_sb[:], wv_d.ap())
                for sl_i in range(1, 4):
                    nc.sync.dma_start(xT_sb[:, sl_i], xT_d.ap()[:, sl_i])
                nc.scalar.dma_start(masks_sb[:], masks_d.ap())
                nc.scalar.dma_start(idb_sb[:], idb_d.ap())

                qkv_t = {}

                def qkv_chunk(b, ci):
                    qT_sb, kT_sb, v_sb = qkv_t[b]
                    cg = b * TC + ci  # global 512-chunk index
                    qk_ps = ps_qk.tile([128, 2, 512], F32, tag="qs",
                                       name=f"qk_{b}_{ci}")
                    for eo in range(EO):
                        nc.tensor.matmul(qk_ps[:, 0, :], wq_sb[:, eo, :],
                                         xT_sb[:, cg, eo, :],
                                         start=eo == 0, stop=eo == EO - 1)
                    for eo in range(EO):
                        nc.tensor.matmul(qk_ps[:, 1, :], wk_sb[:, eo, :],
                                         xT_sb[:, cg, eo, :],
                                         start=eo == 0, stop=eo == EO - 1)
                    nc.vector.tensor_copy(
                        qT_sb[:, 512 * ci:512 * ci + 512], qk_ps[:, 0, :])
                    nc.vector.tensor_copy(
                        kT_sb[:, 512 * ci:512 * ci + 512], qk_ps[:, 1, :])
                    for k2 in range(4):
                        vp = ps_v.tile([128, 128], F32, tag="vtp",
                                       name=f"vp_{b}_{ci}_{k2}")
                        for eo in range(EO):
                            nc.tensor.matmul(
                                vp[:],
                                xT_sb[:, cg, eo, 128 * k2:128 * (k2 + 1)],
                                wv_sb[:, eo, :],
                                start=eo == 0, stop=eo == EO - 1)
                        ts_ = 4 * ci + k2
                        vrow = v_sb[:, ts_, :]
                        # ones columns at 64 and 129
                        ones_view = bass.AP(
                            tensor=vrow.tensor,
                            offset=vrow.offset + HS,
                            ap=[vrow.ap[0], [HS + 1, 2]])
                        nc.vector.memset(ones_view, 1.0)
                        # v columns: psum [128,(2,64)] -> cols {0..63},{65..128}
                        dst = bass.AP(
                            tensor=vrow.tensor, offset=vrow.offset,
                            ap=[vrow.ap[0], [HS + 1, 2], [1, HS]])
                        nc.vector.tensor_copy(
                            dst, vp[:].rearrange("p (h d) -> p h d", h=2))

                def attn_blocks(b, blocks):
                    qT_sb, kT_sb, v_sb = qkv_t[b]
                    with nc.named_scope(f"attn{b}"):
                        for i in blocks:
                            # two banks, each holding accumulators for a pair
                            # of 128-token subtiles: [:, k2%2, h, :]
                            av_ps = [ps_av.tile([128, 2, 2, HS + 1], F32,
                                                tag="av", name=f"av_{b}_{i}_{p}")
                                     for p in range(2)]
                            nj = 4 * i + 4
                            for j in range(nj):
                                s_ps = ps_s.tile([128, 2, 512], F32, tag="qs")
                                for h in range(2):
                                    nc.tensor.matmul(
                                        s_ps[:, h, :],
                                        kT_sb[64 * h:64 * h + 64,
                                              128 * j:128 * j + 128],
                                        qT_sb[64 * h:64 * h + 64,
                                              512 * i:512 * i + 512],
                                        start=True, stop=True)
                                pt = att_pt.tile([128, 2, 512], BF, tag="pt")
                                nc.scalar.activation(pt[:], s_ps[:], AF.Exp,
                                                     scale=1.0 / np.sqrt(HS))
                                if j >= 4 * i:
                                    q = j - 4 * i
                                    nc.vector.tensor_tensor(
                                        pt[:], pt[:],
                                        masks_sb[:, q, None, :].to_broadcast(
                                            (128, 2, 512)),
                                        OP.mult)
                                for k2 in range(4):
                                    if j > 4 * i + k2:
                                        continue
                                    for h in range(2):
                                        # start=True clears has_written for the
                                        # WHOLE bank, so only the very first
                                        # matmul into each bank may set it; the
                                        # other regions rely on per-element
                                        # first-write-overwrite semantics.
                                        nc.tensor.matmul(
                                            av_ps[k2 // 2][:, k2 % 2, h, :],
                                            pt[:, h, 128 * k2:128 * (k2 + 1)],
                                            v_sb[:, j, (HS + 1) * h:(HS + 1) * (h + 1)],
                                            start=(j == 0 and h == 0
                                                   and k2 % 2 == 0),
                                            stop=j == 4 * i + k2)
                            for k2 in range(4):
                                avp = av_ps[k2 // 2][:, k2 % 2, :, :]
                                recip = att_small.tile([128, 2], F32, tag="recip")
                                nc.vector.reciprocal(recip[:], avp[:, :, HS])
                                onorm = att_small.tile([128, 128], BF, tag="onorm")
                                for h in range(2):
                                    nc.vector.tensor_scalar_mul(
                                        onorm[:, 64 * h:64 * h + 64],
                                        avp[:, h, 0:HS],
                                        recip[:, h:h + 1])
                                tp = ps_tp.tile([128, 128], BF, tag="vtp")
                                nc.tensor.transpose(tp[:], onorm[:], idb_sb[:])
                                ot = att_small.tile([128, 128], BF, tag="ot")
                                nc.vector.tensor_copy(ot[:], tp[:])
                                g2_ = 512 * i + 128 * k2  # within-batch col
                                oeng = nc.sync if b == 0 else nc.scalar
                                oeng.dma_start(
                                    a2a_in[b][g2_ // HSL, :,
                                              (g2_ % HSL):(g2_ % HSL) + 128],
                                    ot[:])

                def a2a(b):
                    with nc.named_scope(f"a2a{b}"):
                        nc.gpsimd.collective_compute(
                            "AllToAll", mybir.AluOpType.bypass,
                            replica_groups=[list(range(n_cores))],
                            ins=[a2a_in[b].opt()], outs=[a2a_out[b].opt()])
                    # gather on the sync ring, which carries nothing else at
                    # this point (batch-1 stores are on scalar): the sequencer
                    # blocks on the a2a semaphore, then streams immediately
                    for hh in range(2):
                        nc.sync.dma_start(
                            hcT[b][:, hh * (EO // 2):(hh + 1) * (EO // 2)],
                            a2a_out[b][hh * (EO // 2):(hh + 1) * (EO // 2)]
                            .rearrange("i p t -> p i t"))

                # driver: qkv0 → (attn0 ⊗ qkv1) → a2a0 → attn1 → a2a1
                qkv_t[0] = (
                    att_qkv.tile([128, T], BF, tag="qT", name="qT0"),
                    att_qkv.tile([128, T], BF, tag="kT", name="kT0"),
                    att_qkv.tile([128, T // 128, 2 * (HS + 1)], BF, tag="v",
                                 name="v0"))
                with nc.named_scope("qkv0"):
                    for ci in range(2):
                        qkv_chunk(0, ci)
                    # batch-1 xT chunks gated behind qkv0 chunk-1's vector
                    # work (WAW dep) so the startup DMA engines serve the
                    # critical chunks first
                    for sl_i in range(4, NTOK // 512):
                        nc.vector.memset(xT_sb[:, sl_i, 0, 0:1], 0.0)
                        eng = nc.scalar if sl_i % 2 == 0 else nc.sync
                        eng.dma_start(xT_sb[:, sl_i], xT_d.ap()[:, sl_i])
                    for ci in range(2, TC):
                        qkv_chunk(0, ci)
                qkv_t[1] = (
                    att_qkv.tile([128, T], BF, tag="qT", name="qT1"),
                    att_qkv.tile([128, T], BF, tag="kT", name="kT1"),
                    att_qkv.tile([128, T // 128, 2 * (HS + 1)], BF, tag="v",
                                 name="v1"))
                for i in range(TC):
                    attn_blocks(0, [i])
                    with nc.named_scope("qkv1"):
                        qkv_chunk(1, i)
                a2a(0)
                load_back_half_consts()
                attn_blocks(1, range(TC))
                a2a(1)

            for m_ in range(TSL // 128):
                nc.vector.tensor_tensor(xpb_sb[:, m_, :], xsl_sb[:, m_, :],
                                        bo_bc[:], OP.add)
            # bf16 copies of the LN1 gain/bias for the bf16 apply path
            g1_b16 = consts.tile([128, E], BF)
            nc.vector.tensor_copy(g1_b16[:], g1_bc[:])
            be1_b16 = consts.tile([128, E], BF)
            nc.vector.tensor_copy(be1_b16[:], be1_bc[:])

            # ================= back half (2 x 256-token half-slices) =====
            with tc.tile_pool(name="bh", bufs=1) as bh, \
                 tc.tile_pool(name="bh_w1", bufs=8) as bh_w1, \
                 tc.tile_pool(name="bh_w2", bufs=4) as bh_w2, \
                 tc.tile_pool(name="bh_small", bufs=4) as bh_small:

                x1f = bh.tile([128, TSL // 128, E], F32, tag="x1f")
                x1n = bh.tile([128, TSL // 128, E], BF, tag="x1n")
                x1b = bh.tile([128, TSL // 128, E], F32, tag="x1b")
                x1T = bh.tile([128, EO, TSL], F8, tag="x1T")
                hT = bh.tile([128, FO, TSL], BF, tag="hT")
                out_dst = out_d.ap().rearrange("(m p) e -> p m e", p=128)

                def layernorm(buf_m, g_bc, be_bc):
                    # in-place LN over the last (1024) axis of buf_m [128, E]
                    stats = bh_small.tile([128, 2, 6], F32, tag="stats")
                    for s2 in range(2):
                        nc.vector.bn_stats(stats[:, s2, :],
                                           buf_m[:, 512 * s2:512 * (s2 + 1)])
                    mv = bh_small.tile([128, 2], F32, tag="mv")
                    nc.vector.bn_aggr(mv[:], stats[:])
                    std = bh_small.tile([128, 1], F32, tag="std")
                    nc.scalar.activation(std[:], mv[:, 1:2], AF.Sqrt,
                                         bias=eps_sb[:, 0:1])
                    rstd = bh_small.tile([128, 1], F32, tag="rstd")
                    nc.vector.reciprocal(rstd[:], std[:])
                    nc.vector.tensor_scalar(
                        buf_m[:], buf_m[:], mv[:, 0:1], rstd[:],
                        op0=OP.subtract, op1=OP.mult)
                    nc.vector.tensor_tensor(buf_m[:], buf_m[:], g_bc[:], OP.mult)
                    nc.vector.tensor_tensor(buf_m[:], buf_m[:], be_bc[:], OP.add)

                M2N = HSL // 128  # 2 row-subtiles per half
                with tc.tile_pool(name="ps_wo", bufs=2, space="PSUM") as ps_wo, \
                     tc.tile_pool(name="ps_tp2", bufs=2, space="PSUM") as ps_tp2:
                    def wo_half(h2):
                        with nc.named_scope(f"wo_ln1_{h2}"):
                            for m2 in range(M2N):
                                m = M2N * h2 + m2
                                for n in range(2):
                                    wo_ps = ps_wo.tile([128, 512], F32, tag="wo")
                                    for ho in range(EO):
                                        nc.tensor.matmul(
                                            wo_ps[:],
                                            hcT[h2][:, ho, 128 * m2:128 * (m2 + 1)],
                                            wo_sb[:, ho, 512 * n:512 * (n + 1)],
                                            start=ho == 0, stop=ho == EO - 1)
                                    sl = slice(512 * n, 512 * (n + 1))
                                    nc.vector.tensor_tensor(
                                        x1f[:, m, sl], wo_ps[:], xpb_sb[:, m, sl],
                                        OP.add)
                                # LN1: stats in f32, apply into bf16 (halves
                                # DVE cost on this latency-critical stretch)
                                stats = bh_small.tile([128, 2, 6], F32,
                                                      tag="stats")
                                for s2 in range(2):
                                    nc.vector.bn_stats(
                                        stats[:, s2, :],
                                        x1f[:, m, 512 * s2:512 * (s2 + 1)])
                                mv = bh_small.tile([128, 2], F32, tag="mv")
                                nc.vector.bn_aggr(mv[:], stats[:])
                                std = bh_small.tile([128, 1], F32, tag="std")
                                nc.scalar.activation(std[:], mv[:, 1:2],
                                                     AF.Sqrt,
                                                     bias=eps_sb[:, 0:1])
                                rstd = bh_small.tile([128, 1], F32, tag="rstd")
                                nc.vector.reciprocal(rstd[:], std[:])
                                nc.vector.tensor_scalar(
                                    x1n[:, m, :], x1f[:, m, :], mv[:, 0:1],
                                    rstd[:], op0=OP.subtract, op1=OP.mult)
                                nc.vector.tensor_tensor(
                                    x1n[:, m, :], x1n[:, m, :], g1_b16[:],
                                    OP.mult)
                                nc.vector.tensor_tensor(
                                    x1n[:, m, :], x1n[:, m, :], be1_b16[:],
                                    OP.add)
                                for eo in range(EO):
                                    tp2 = ps_tp2.tile([128, 128], BF,
                                                      tag="tp2")
                                    nc.tensor.transpose(
                                        tp2[:], x1n[:, m, 128 * eo:128 * (eo + 1)],
                                        idb_sb[:])
                                    nc.vector.tensor_copy(
                                        x1T[:, eo, 128 * m:128 * (m + 1)], tp2[:])

                    def ffn1_range(fos, half, ps_f1):
                        # half: None = full token width, 0/1 = 256-token half
                        t0, t1 = ((0, TSL) if half is None
                                  else (HSL * half, HSL * (half + 1)))
                        fol = list(fos)
                        for fp in range(len(fol) // 2):
                            fo0 = fol[2 * fp]
                            w1t = bh_w1.tile([128, 2, EO // 2, 256], F8,
                                             tag="w1t")
                            nc.scalar.dma_start(w1t[:],
                                                w1_d.ap()[:, fo0:fo0 + 2])
                            for f2i in range(2):
                                fo = fo0 + f2i
                                f1_ps = ps_f1.tile([128, TSL], F32, tag="f1")
                                for q4 in range(EO // 2):
                                    nc.tensor.matmul(
                                        f1_ps[:, 0:t1 - t0],
                                        w1t[:, f2i, q4].rearrange(
                                            "p (two m) -> p two m", two=2),
                                        x1T[:, 2 * q4:2 * q4 + 2, t0:t1],
                                        start=q4 == 0,
                                        stop=q4 == EO // 2 - 1,
                                        perf_mode=DRSI)
                                nc.scalar.activation(hT[:, fo, t0:t1],
                                                     f1_ps[:, 0:t1 - t0],
                                                     AF.Relu,
                                                     bias=b1_sb[:, fo:fo + 1])

                    # wo/LN1 for the batch-0 half, then bridge the a2a1 +
                    # gather latency with ffn1 work on the finished half
                    FSPLIT = 20
                    wo_half(0)
                    with nc.named_scope("ffn1"), \
                         tc.tile_pool(name="ps_f1", bufs=3,
                                      space="PSUM") as ps_f1:
                        for m in range(2):
                            nc.vector.tensor_tensor(
                                x1b[:, m, :], x1n[:, m, :], b2_bc[:], OP.add)
                        ffn1_range(range(FSPLIT), 0, ps_f1)
                        wo_half(1)
                        for m in range(2, 4):
                            nc.vector.tensor_tensor(
                                x1b[:, m, :], x1n[:, m, :], b2_bc[:], OP.add)
                        ffn1_range(range(FSPLIT), 1, ps_f1)
                        ffn1_range(range(FSPLIT, FO), None, ps_f1)

                # ffn2 in two m-group passes (W2 streamed twice) so the
                # LN2/store epilogue of pass 0 overlaps pass 1's matmuls
                with nc.named_scope("ffn2_ln2"), \
                     tc.tile_pool(name="ps_f2", bufs=8, space="PSUM") as ps_f2:
                    w2r = w2_d.ap().rearrange("(fo p) e -> p fo e", p=128)
                    for g in range(2):
                        f2_ps = [ps_f2.tile([128, 512], F32, tag="f2",
                                            name=f"f2_{g}_{k}")
                                 for k in range(4)]
                        for fp in range(FO // 2):  # paired w2 loads
                            w2t = bh_w2.tile([128, 2, E], BF, tag="w2t")
                            eng = nc.scalar if fp % 2 == 0 else nc.sync
                            eng.dma_start(w2t[:], w2r[:, 2 * fp:2 * fp + 2, :])
                            for f2i in range(2):
                                fo = 2 * fp + f2i
                                for mi in range(2):
                                    m = 2 * g + mi
                                    for n in range(2):
                                        nc.tensor.matmul(
                                            f2_ps[2 * mi + n][:],
                                            hT[:, fo, 128 * m:128 * (m + 1)],
                                            w2t[:, f2i, 512 * n:512 * (n + 1)],
                                            start=fo == 0, stop=fo == FO - 1)
                        for mi in range(2):
                            m = 2 * g + mi
                            for n in range(2):
                                sl = slice(512 * n, 512 * (n + 1))
                                nc.vector.tensor_tensor(
                                    x1b[:, m, sl], f2_ps[2 * mi + n][:],
                                    x1b[:, m, sl], OP.add)
                            layernorm(x1b[:, m, :], g2_bc, be2_bc)
                            nc.sync.dma_start(out_dst[:, m, :], x1b[:, m, :])

    nc.compile()
    return nc


def _make_in_maps(inputs):
    x = np.asarray(inputs["x"], dtype=np.float32)
    Wq = np.asarray(inputs["Wq"], dtype=np.float32)
    Wk = np.asarray(inputs["Wk"], dtype=np.float32)
    Wv = np.asarray(inputs["Wv"], dtype=np.float32)
    Wo = np.asarray(inputs["Wo"], dtype=np.float32)

    xflat = x.reshape(NTOK, E)
    # [p, t-chunk, eo, t'] — one contiguous 8KB run per partition per chunk
    xT = np.ascontiguousarray(
        xflat.reshape(NTOK // 512, 512, EO_, 128).transpose(3, 0, 2, 1)
    ).astype(BF16)
    wo = np.ascontiguousarray(
        Wo.reshape(EO_, 128, E).transpose(1, 0, 2)).astype(BF16)
    w1b = np.ascontiguousarray(
        np.asarray(inputs["W1"], dtype=np.float32)
        .reshape(EO_, 128, 4 * E // 128, 128).transpose(1, 2, 0, 3)
    ).astype(F8E4)  # [p, fo, eo, hid]
    # interleave eo-pairs and reverse the hid columns for SwInterleave:
    # flat[2j + i] = w1b[p, fo, 2*q4 + i, 127 - j]
    w1 = np.ascontiguousarray(
        w1b.reshape(128, 4 * E // 128, EO_ // 2, 2, 128)[..., ::-1]
        .transpose(0, 1, 2, 4, 3)
        .reshape(128, 4 * E // 128, EO_ // 2, 256))
    w2 = np.asarray(inputs["W2"], dtype=np.float32).astype(BF16)
    b1s = np.ascontiguousarray(
        np.asarray(inputs["b1"], dtype=np.float32).reshape(4 * E // 128, 128).T)

    # mask patterns for the 4 diagonal-straddling [s=128, t=512] tiles,
    # pre-tiled to [p, q, t]
    masks4 = np.zeros((4, 128, 512), dtype=np.float32)
    srow = np.arange(128)[:, None]
    tcol = np.arange(512)[None, :]
    for q_ in range(4):
        masks4[q_] = (srow <= tcol - 128 * q_)
    masks = np.ascontiguousarray(masks4.transpose(1, 0, 2)).astype(BF16)

    ident = np.eye(128, dtype=np.float32)

    common = {
        "xT": xT,
        "wo": wo,
        "w1": w1,
        "w2": w2,
        "b1s": b1s,
        "bo": np.asarray(inputs["bo"], dtype=np.float32),
        "b1": np.asarray(inputs["b1"], dtype=np.float32),
        "b2": np.asarray(inputs["b2"], dtype=np.float32),
        "g1": np.asarray(inputs["g1"], dtype=np.float32),
        "be1": np.asarray(inputs["be1"], dtype=np.float32),
        "g2": np.asarray(inputs["g2"], dtype=np.float32),
        "be2": np.asarray(inputs["be2"], dtype=np.float32),
        "masks": masks,
        "id_bf": ident.astype(BF16),
    }
    in_maps = []
    for c in range(N_CORES):
        m = dict(common)
        def tile_w(W):
            # [E, 128] head-pair concat -> [p, eo, d]
            wc = np.concatenate([W[2 * c], W[2 * c + 1]], axis=1)
            return np.ascontiguousarray(
                wc.reshape(EO_, 128, 128).transpose(1, 0, 2)).astype(BF16)
        m["wq"] = tile_w(Wq)
        m["wk"] = tile_w(Wk)
        m["wv"] = tile_w(Wv)
        # per-core token rows: batch-0 [256c, 256c+256) then batch-1 same,
        # pre-tiled to [p, m, e]
        rows = np.concatenate([
            xflat[HSL * c:HSL * (c + 1)],
            xflat[T + HSL * c:T + HSL * (c + 1)]], axis=0)
        m["x_slice"] = np.ascontiguousarray(
            rows.reshape(TSL // 128, 128, E).transpose(1, 0, 2))
        in_maps.append(m)
    return in_maps


def _enable_trace_hook():
    """Register the axon NTFF profile hook (missing antenv.axon_hooks shim)."""
    import types
    import antenv  # noqa: F401

    if "antenv.axon_hooks" not in sys.modules:
        mod = types.ModuleType("antenv.axon_hooks")
        _hook = [None]
        mod.set_axon_ntff_profile_hook = lambda h: _hook.__setitem__(0, h)
        mod.get_axon_ntff_profile_hook = lambda: _hook[0]
        sys.modules["antenv.axon_hooks"] = mod
        antenv.axon_hooks = mod
    mod = sys.modules["antenv.axon_hooks"]
    if mod.get_axon_ntff_profile_hook() is None:
        from trn_agent_boot.trn_boot import _ntff_profile_via_ctypes
        mod.set_axon_ntff_profile_hook(
            _ntff_profile_via_ctypes("/opt/axon/libaxon_pjrt.so"))


def run(inputs, trace=False):
    """Returns (full_output [B,T,E] f32, BassKernelResults)."""
    from concourse import bass_utils

    if "nc" not in _cache:
        _cache["nc"] = _build()
    nc = _cache["nc"]
    in_maps = _make_in_maps(inputs)
    if trace:
        _enable_trace_hook()
    res = bass_utils.run_bass_kernel_spmd(
        nc, in_maps, core_ids=list(range(N_CORES)), trace=trace)
    full = np.empty((NTOK, E), dtype=np.float32)
    for c in range(N_CORES):
        o = res.results[c]["out"]
        full[HSL * c:HSL * (c + 1)] = o[:HSL]
        full[T + HSL * c:T + HSL * (c + 1)] = o[HSL:]
    return full.reshape(B, T, E), res


def kernel(**inputs):
    out, _ = run(inputs, trace=False)
    return out



# revision 49
# speedup vs baseline: 1.0124x; 1.0124x over previous
"""Trainium2 Bass kernel for a dense transformer block (causal MHA + FFN, post-LN).

Sharding over 8 NeuronCores:
  - Attention is tensor-parallel over heads: core c computes heads 2c, 2c+1
    for all 4096 tokens (B*T flattened, batch-major).
  - One AllToAll per batch redistributes the per-head attention outputs so
    core c ends up with the full head-concatenated attention output
    (transposed) for its token half-slices: batch-0 tokens [256c, 256c+256)
    and batch-1 tokens [256c, 256c+256).
  - Wo + residual + LN1 + FFN + residual + LN2 are sequence-parallel: each
    core processes its 2x256 token rows and outputs [512, 1024].

Matmuls run in bf16 (fp32 PSUM accumulation), except ffn1 which runs in fp8
e4m3 DoubleRowSwInterleave (2 contraction rows per cycle; W1 and x1T stored
as fp8). Softmax runs without the max subtraction (score range is +-2 here,
exp cannot overflow), with the softmax denominator obtained for free as an
extra ones-column in the P@V matmul. Residuals / layernorms are fp32; the
LN1 apply is bf16.

Overlap structure: batch-1 qkv chunks are interleaved into batch-0's
attention i-blocks; the back-half constants stream after the a2a0 dispatch;
wo/LN1 for batch-0 rows starts as soon as attn1 drains (its gather is issued
on two DMA rings), and the first 20 fo's of ffn1 run on the finished token
half to bridge the a2a1 + gather latency before wo/LN1 of batch-1 rows; ffn2
runs in two m-group passes so the LN2/store epilogue overlaps pass-1 matmuls.
"""

import sys

sys.path.insert(0, "/opt/trn_rl_repo")

import numpy as np
import ml_dtypes

B, T, E, H = 2, 2048, 1024, 16
HS = E // H  # 64
N_CORES = 8
HPC = H // N_CORES  # heads per core = 2
NTOK = B * T  # 4096
TSL = NTOK // N_CORES  # 512 token rows per core
HSL = TSL // B  # 256 rows per (core, batch) half-slice
EPS = 1e-5

BF16 = ml_dtypes.bfloat16
F8E4 = ml_dtypes.float8_e4m3fn
EO_ = E // 128

_cache = {}


def _build(n_cores=N_CORES):
    import concourse.bass as bass
    import concourse.tile as tile
    import concourse.bacc as bacc
    from concourse import mybir

    BF = mybir.dt.bfloat16
    F32 = mybir.dt.float32
    F8 = mybir.dt.float8e4
    DR = mybir.MatmulPerfMode.DoubleRow
    DRSI = mybir.MatmulPerfMode.DoubleRowSwInterleave
    AF = mybir.ActivationFunctionType
    OP = mybir.AluOpType

    nc = bacc.Bacc("TRN2", target_bir_lowering=False, debug=False,
                   num_devices=n_cores)

    EO = E // 128            # 8 chunks of the embedding dim
    FO = 4 * E // 128        # 32 chunks of the FFN hidden dim
    TC = T // 512            # 4 t-chunks of 512 per batch

    # ---- I/O (host passes pre-tiled layouts: 1 contiguous run/partition) --
    xT_d = nc.dram_tensor("xT", [128, NTOK // 512, EO, 512], BF,
                          kind="ExternalInput")
    xsl_d = nc.dram_tensor("x_slice", [128, TSL // 128, E], F32,
                           kind="ExternalInput")
    wq_d = nc.dram_tensor("wq", [128, E // 128, HPC * HS], BF,
                          kind="ExternalInput")
    wk_d = nc.dram_tensor("wk", [128, E // 128, HPC * HS], BF,
                          kind="ExternalInput")
    wv_d = nc.dram_tensor("wv", [128, E // 128, HPC * HS], BF,
                          kind="ExternalInput")
    wo_d = nc.dram_tensor("wo", [128, EO, E], BF, kind="ExternalInput")
    # W1 pre-interleaved for DoubleRowSwInterleave: per (fo, eo-pair) the 256
    # columns are [A127, B127, A126, ..., B0] (pair-interleaved, reversed)
    w1_d = nc.dram_tensor("w1", [128, 4 * E // 128, EO // 2, 256], F8,
                          kind="ExternalInput")
    w2_d = nc.dram_tensor("w2", [4 * E, E], BF, kind="ExternalInput")
    b1s_d = nc.dram_tensor("b1s", [128, 4 * E // 128], F32,
                           kind="ExternalInput")
    bo_d = nc.dram_tensor("bo", [E], F32, kind="ExternalInput")
    b1_d = nc.dram_tensor("b1", [4 * E], F32, kind="ExternalInput")
    b2_d = nc.dram_tensor("b2", [E], F32, kind="ExternalInput")
    g1_d = nc.dram_tensor("g1", [E], F32, kind="ExternalInput")
    be1_d = nc.dram_tensor("be1", [E], F32, kind="ExternalInput")
    g2_d = nc.dram_tensor("g2", [E], F32, kind="ExternalInput")
    be2_d = nc.dram_tensor("be2", [E], F32, kind="ExternalInput")
    masks_d = nc.dram_tensor("masks", [128, 4, 512], BF, kind="ExternalInput")
    idb_d = nc.dram_tensor("id_bf", [128, 128], BF, kind="ExternalInput")
    out_d = nc.dram_tensor("out", [TSL, E], F32, kind="ExternalOutput")

    def bcast_ap(d, n):
        # [n]-vector in DRAM broadcast across 128 partitions
        a = d.ap()
        return bass.AP(tensor=a.tensor, offset=a.offset, ap=[[0, 128], [1, n]])

    with tile.TileContext(nc) as tc:
        with tc.tile_pool(name="dram", bufs=1, space="DRAM") as dram, \
             tc.tile_pool(name="consts", bufs=1) as consts:

            a2a_in = [dram.tile([n_cores, 128, HSL], BF, name=f"a2a_in{b}")
                      for b in range(B)]
            a2a_out = [dram.tile([n_cores, 128, HSL], BF, name=f"a2a_out{b}")
                       for b in range(B)]

            # ---- attention-critical constants first ---------------------
            wq_sb = consts.tile([128, EO, HPC * HS], BF)
            nc.scalar.dma_start(wq_sb[:], wq_d.ap())
            wk_sb = consts.tile([128, EO, HPC * HS], BF)
            wv_sb = consts.tile([128, EO, HPC * HS], BF)
            masks_sb = consts.tile([128, 4, 512], BF)
            idb_sb = consts.tile([128, 128], BF)
            eps_sb = consts.tile([128, 1], F32)
            nc.vector.memset(eps_sb[:], EPS)
            # back-half constants: tiles allocated here, but their DMAs are
            # deferred until after the a2a0 dispatch so the startup DMA
            # engines are fully available for the xT stream
            xsl_sb = consts.tile([128, TSL // 128, E], F32)
            wo_sb = consts.tile([128, EO, E], BF)
            b1_sb = consts.tile([128, FO], F32)
            bo_bc = consts.tile([128, E], F32)
            b2_bc = consts.tile([128, E], F32)
            g1_bc = consts.tile([128, E], F32)
            be1_bc = consts.tile([128, E], F32)
            g2_bc = consts.tile([128, E], F32)
            be2_bc = consts.tile([128, E], F32)

            def load_back_half_consts():
                nc.gpsimd.dma_start(xsl_sb[:], xsl_d.ap())
                nc.gpsimd.dma_start(wo_sb[:], wo_d.ap())
                nc.gpsimd.dma_start(b1_sb[:], b1s_d.ap())
                nc.gpsimd.dma_start(bo_bc[:], bcast_ap(bo_d, E))
                nc.gpsimd.dma_start(b2_bc[:], bcast_ap(b2_d, E))
                nc.gpsimd.dma_start(g1_bc[:], bcast_ap(g1_d, E))
                nc.gpsimd.dma_start(be1_bc[:], bcast_ap(be1_d, E))
                nc.gpsimd.dma_start(g2_bc[:], bcast_ap(g2_d, E))
                nc.gpsimd.dma_start(be2_bc[:], bcast_ap(be2_d, E))
            # x + bo precomputed once; saves one DVE add per Wo psum tile
            # (computed after the attention loop so it can't stall qkv copies)
            xpb_sb = consts.tile([128, TSL // 128, E], F32)

            # persistent home for the post-a2a gathered attention output so
            # its DMA can be issued right after each collective
            hcT = [consts.tile([128, EO, HSL], BF, tag=f"hcT{b}",
                               name=f"hcT{b}")
                   for b in range(B)]

            # ================= attention (heads 2c, 2c+1) =================
            with tc.tile_pool(name="att_big", bufs=1) as att_big, \
                 tc.tile_pool(name="att_qkv", bufs=2) as att_qkv, \
                 tc.tile_pool(name="att_pt", bufs=4) as att_pt, \
                 tc.tile_pool(name="att_small", bufs=4) as att_small, \
                 tc.tile_pool(name="ps_big", bufs=2, space="PSUM") as ps_big, \
                 tc.tile_pool(name="ps_small", bufs=2, space="PSUM") as ps_small, \
                 tc.tile_pool(name="ps_av", bufs=2, space="PSUM") as ps_av:
                ps_qk = ps_s = ps_big          # share 2x 2-bank slots (tag "qs")
                ps_v = ps_tp = ps_small        # share 2x 1-bank slots (tag "vtp")

                # [p, t-chunk, eo, 512] — slice-major so one DMA per t-chunk
                # is a single contiguous 8KB run per partition; chunks split
                # across the sync + scalar HWDGE rings so qkv0 never starves
                xT_sb = att_big.tile([128, NTOK // 512, EO, 512], BF, tag="xT")
                nc.sync.dma_start(xT_sb[:, 0], xT_d.ap()[:, 0])
                nc.scalar.dma_start(wk_sb[:], wk_d.ap())
                nc.scalar.dma_start(wv_sb[:], wv_d.ap())
                for sl_i in range(1, 4):
                    nc.sync.dma_start(xT_sb[:, sl_i], xT_d.ap()[:, sl_i])
                nc.scalar.dma_start(masks_sb[:], masks_d.ap())
                nc.scalar.dma_start(idb_sb[:], idb_d.ap())

                qkv_t = {}

                def qkv_chunk(b, ci):
                    qT_sb, kT_sb, v_sb = qkv_t[b]
                    cg = b * TC + ci  # global 512-chunk index
                    qk_ps = ps_qk.tile([128, 2, 512], F32, tag="qs",
                                       name=f"qk_{b}_{ci}")
                    for eo in range(EO):
                        nc.tensor.matmul(qk_ps[:, 0, :], wq_sb[:, eo, :],
                                         xT_sb[:, cg, eo, :],
                                         start=eo == 0, stop=eo == EO - 1)
                    for eo in range(EO):
                        nc.tensor.matmul(qk_ps[:, 1, :], wk_sb[:, eo, :],
                                         xT_sb[:, cg, eo, :],
                                         start=eo == 0, stop=eo == EO - 1)
                    nc.vector.tensor_copy(
                        qT_sb[:, 512 * ci:512 * ci + 512], qk_ps[:, 0, :])
                    nc.vector.tensor_copy(
                        kT_sb[:, 512 * ci:512 * ci + 512], qk_ps[:, 1, :])
                    for k2 in range(4):
                        vp = ps_v.tile([128, 128], F32, tag="vtp",
                                       name=f"vp_{b}_{ci}_{k2}")
                        for eo in range(EO):
                            nc.tensor.matmul(
                                vp[:],
                                xT_sb[:, cg, eo, 128 * k2:128 * (k2 + 1)],
                                wv_sb[:, eo, :],
                                start=eo == 0, stop=eo == EO - 1)
                        ts_ = 4 * ci + k2
                        vrow = v_sb[:, ts_, :]
                        # ones columns at 64 and 129
                        ones_view = bass.AP(
                            tensor=vrow.tensor,
                            offset=vrow.offset + HS,
                            ap=[vrow.ap[0], [HS + 1, 2]])
                        nc.vector.memset(ones_view, 1.0)
                        # v columns: psum [128,(2,64)] -> cols {0..63},{65..128}
                        dst = bass.AP(
                            tensor=vrow.tensor, offset=vrow.offset,
                            ap=[vrow.ap[0], [HS + 1, 2], [1, HS]])
                        nc.vector.tensor_copy(
                            dst, vp[:].rearrange("p (h d) -> p h d", h=2))

                def attn_blocks(b, blocks):
                    qT_sb, kT_sb, v_sb = qkv_t[b]
                    with nc.named_scope(f"attn{b}"):
                        for i in blocks:
                            # two banks, each holding accumulators for a pair
                            # of 128-token subtiles: [:, k2%2, h, :]
                            av_ps = [ps_av.tile([128, 2, 2, HS + 1], F32,
                                                tag="av", name=f"av_{b}_{i}_{p}")
                                     for p in range(2)]
                            nj = 4 * i + 4
                            for j in range(nj):
                                s_ps = ps_s.tile([128, 2, 512], F32, tag="qs")
                                for h in range(2):
                                    nc.tensor.matmul(
                                        s_ps[:, h, :],
                                        kT_sb[64 * h:64 * h + 64,
                                              128 * j:128 * j + 128],
                                        qT_sb[64 * h:64 * h + 64,
                                              512 * i:512 * i + 512],
                                        start=True, stop=True)
                                pt = att_pt.tile([128, 2, 512], BF, tag="pt")
                                nc.scalar.activation(pt[:], s_ps[:], AF.Exp,
                                                     scale=1.0 / np.sqrt(HS))
                                if j >= 4 * i:
                                    q = j - 4 * i
                                    nc.vector.tensor_tensor(
                                        pt[:], pt[:],
                                        masks_sb[:, q, None, :].to_broadcast(
                                            (128, 2, 512)),
                                        OP.mult)
                                for k2 in range(4):
                                    if j > 4 * i + k2:
                                        continue
                                    for h in range(2):
                                        # start=True clears has_written for the
                                        # WHOLE bank, so only the very first
                                        # matmul into each bank may set it; the
                                        # other regions rely on per-element
                                        # first-write-overwrite semantics.
                                        nc.tensor.matmul(
                                            av_ps[k2 // 2][:, k2 % 2, h, :],
                                            pt[:, h, 128 * k2:128 * (k2 + 1)],
                                            v_sb[:, j, (HS + 1) * h:(HS + 1) * (h + 1)],
                                            start=(j == 0 and h == 0
                                                   and k2 % 2 == 0),
                                            stop=j == 4 * i + k2)
                            for k2 in range(4):
                                avp = av_ps[k2 // 2][:, k2 % 2, :, :]
                                recip = att_small.tile([128, 2], F32, tag="recip")
                                nc.vector.reciprocal(recip[:], avp[:, :, HS])
                                onorm = att_small.tile([128, 128], BF, tag="onorm")
                                for h in range(2):
                                    nc.vector.tensor_scalar_mul(
                                        onorm[:, 64 * h:64 * h + 64],
                                        avp[:, h, 0:HS],
                                        recip[:, h:h + 1])
                                tp = ps_tp.tile([128, 128], BF, tag="vtp")
                                nc.tensor.transpose(tp[:], onorm[:], idb_sb[:])
                                ot = att_small.tile([128, 128], BF, tag="ot")
                                nc.vector.tensor_copy(ot[:], tp[:])
                                g2_ = 512 * i + 128 * k2  # within-batch col
                                oeng = nc.sync if b == 0 else nc.scalar
                                oeng.dma_start(
                                    a2a_in[b][g2_ // HSL, :,
                                              (g2_ % HSL):(g2_ % HSL) + 128],
                                    ot[:])

                def a2a(b):
                    with nc.named_scope(f"a2a{b}"):
                        nc.gpsimd.collective_compute(
                            "AllToAll", mybir.AluOpType.bypass,
                            replica_groups=[list(range(n_cores))],
                            ins=[a2a_in[b].opt()], outs=[a2a_out[b].opt()])
                    # gather on the sync ring, which carries nothing else at
                    # this point (batch-1 stores are on scalar): the sequencer
                    # blocks on the a2a semaphore, then streams immediately
                    for hh in range(2):
                        nc.sync.dma_start(
                            hcT[b][:, hh * (EO // 2):(hh + 1) * (EO // 2)],
                            a2a_out[b][hh * (EO // 2):(hh + 1) * (EO // 2)]
                            .rearrange("i p t -> p i t"))

                # driver: qkv0 → (attn0 ⊗ qkv1) → a2a0 → attn1 → a2a1
                qkv_t[0] = (
                    att_qkv.tile([128, T], BF, tag="qT", name="qT0"),
                    att_qkv.tile([128, T], BF, tag="kT", name="kT0"),
                    att_qkv.tile([128, T // 128, 2 * (HS + 1)], BF, tag="v",
                                 name="v0"))
                with nc.named_scope("qkv0"):
                    for ci in range(2):
                        qkv_chunk(0, ci)
                    # batch-1 xT chunks gated behind qkv0 chunk-1's vector
                    # work (WAW dep) so the startup DMA engines serve the
                    # critical chunks first
                    for sl_i in range(4, NTOK // 512):
                        nc.vector.memset(xT_sb[:, sl_i, 0, 0:1], 0.0)
                        eng = nc.scalar if sl_i % 2 == 0 else nc.sync
                        eng.dma_start(xT_sb[:, sl_i], xT_d.ap()[:, sl_i])
                    for ci in range(2, TC):
                        qkv_chunk(0, ci)
                qkv_t[1] = (
                    att_qkv.tile([128, T], BF, tag="qT", name="qT1"),
                    att_qkv.tile([128, T], BF, tag="kT", name="kT1"),
                    att_qkv.tile([128, T // 128, 2 * (HS + 1)], BF, tag="v",
                                 name="v1"))
                for i in range(TC):
                    attn_blocks(0, [i])
                    with nc.named_scope("qkv1"):
                        qkv_chunk(1, i)
                a2a(0)
                load_back_half_consts()
                attn_blocks(1, range(TC))
                a2a(1)

            for m_ in range(TSL // 128):
                nc.vector.tensor_tensor(xpb_sb[:, m_, :], xsl_sb[:, m_, :],
                                        bo_bc[:], OP.add)
            # bf16 copies of the LN1 gain/bias for the bf16 apply path
            g1_b16 = consts.tile([128, E], BF)
            nc.vector.tensor_copy(g1_b16[:], g1_bc[:])
            be1_b16 = consts.tile([128, E], BF)
            nc.vector.tensor_copy(be1_b16[:], be1_bc[:])

            # ================= back half (2 x 256-token half-slices) =====
            with tc.tile_pool(name="bh", bufs=1) as bh, \
                 tc.tile_pool(name="bh_w1", bufs=8) as bh_w1, \
                 tc.tile_pool(name="bh_w2", bufs=4) as bh_w2, \
                 tc.tile_pool(name="bh_small", bufs=4) as bh_small:

                x1f = bh.tile([128, TSL // 128, E], F32, tag="x1f")
                x1n = bh.tile([128, TSL // 128, E], BF, tag="x1n")
                x1b = bh.tile([128, TSL // 128, E], F32, tag="x1b")
                x1T = bh.tile([128, EO, TSL], F8, tag="x1T")
                hT = bh.tile([128, FO, TSL], BF, tag="hT")
                out_dst = out_d.ap().rearrange("(m p) e -> p m e", p=128)

                def layernorm(buf_m, g_bc, be_bc):
                    # in-place LN over the last (1024) axis of buf_m [128, E]
                    stats = bh_small.tile([128, 2, 6], F32, tag="stats")
                    for s2 in range(2):
                        nc.vector.bn_stats(stats[:, s2, :],
                                           buf_m[:, 512 * s2:512 * (s2 + 1)])
                    mv = bh_small.tile([128, 2], F32, tag="mv")
                    nc.vector.bn_aggr(mv[:], stats[:])
                    std = bh_small.tile([128, 1], F32, tag="std")
                    nc.scalar.activation(std[:], mv[:, 1:2], AF.Sqrt,
                                         bias=eps_sb[:, 0:1])
                    rstd = bh_small.tile([128, 1], F32, tag="rstd")
                    nc.vector.reciprocal(rstd[:], std[:])
                    nc.vector.tensor_scalar(
                        buf_m[:], buf_m[:], mv[:, 0:1], rstd[:],
                        op0=OP.subtract, op1=OP.mult)
                    nc.vector.tensor_tensor(buf_m[:], buf_m[:], g_bc[:], OP.mult)
                    nc.vector.tensor_tensor(buf_m[:], buf_m[:], be_bc[:], OP.add)

                M2N = HSL // 128  # 2 row-subtiles per half
                with tc.tile_pool(name="ps_wo", bufs=2, space="PSUM") as ps_wo, \
                     tc.tile_pool(name="ps_tp2", bufs=2, space="PSUM") as ps_tp2:
                    def wo_half(h2):
                        with nc.named_scope(f"wo_ln1_{h2}"):
                            for m2 in range(M2N):
                                m = M2N * h2 + m2
                                for n in range(2):
                                    wo_ps = ps_wo.tile([128, 512], F32, tag="wo")
                                    for ho in range(EO):
                                        nc.tensor.matmul(
                                            wo_ps[:],
                                            hcT[h2][:, ho, 128 * m2:128 * (m2 + 1)],
                                            wo_sb[:, ho, 512 * n:512 * (n + 1)],
                                            start=ho == 0, stop=ho == EO - 1)
                                    sl = slice(512 * n, 512 * (n + 1))
                                    nc.vector.tensor_tensor(
                                        x1f[:, m, sl], wo_ps[:], xpb_sb[:, m, sl],
                                        OP.add)
                                # LN1: stats in f32, apply into bf16 (halves
                                # DVE cost on this latency-critical stretch)
                                stats = bh_small.tile([128, 2, 6], F32,
                                                      tag="stats")
                                for s2 in range(2):
                                    nc.vector.bn_stats(
                                        stats[:, s2, :],
                                        x1f[:, m, 512 * s2:512 * (s2 + 1)])
                                mv = bh_small.tile([128, 2], F32, tag="mv")
                                nc.vector.bn_aggr(mv[:], stats[:])
                                std = bh_small.tile([128, 1], F32, tag="std")
                                nc.scalar.activation(std[:], mv[:, 1:2],
                                                     AF.Sqrt,
                                                     bias=eps_sb[:, 0:1])
                                rstd = bh_small.tile([128, 1], F32, tag="rstd")
                                nc.vector.reciprocal(rstd[:], std[:])
                                nc.vector.tensor_scalar(
                                    x1n[:, m, :], x1f[:, m, :], mv[:, 0:1],
                                    rstd[:], op0=OP.subtract, op1=OP.mult)
                                nc.vector.tensor_tensor(
                                    x1n[:, m, :], x1n[:, m, :], g1_b16[:],
                                    OP.mult)
                                nc.vector.tensor_tensor(
                                    x1n[:, m, :], x1n[:, m, :], be1_b16[:],
                                    OP.add)
                                for eo in range(EO):
                                    tp2 = ps_tp2.tile([128, 128], BF,
                                                      tag="tp2")
                                    nc.tensor.transpose(
                                        tp2[:], x1n[:, m, 128 * eo:128 * (eo + 1)],
                                        idb_sb[:])
                                    nc.vector.tensor_copy(
                                        x1T[:, eo, 128 * m:128 * (m + 1)], tp2[:])

                    def ffn1_range(fos, half, ps_f1):
                        # half: None = full token width, 0/1 = 256-token half
                        t0, t1 = ((0, TSL) if half is None
                                  else (HSL * half, HSL * (half + 1)))
                        fol = list(fos)
                        for fp in range(len(fol) // 2):
                            fo0 = fol[2 * fp]
                            w1t = bh_w1.tile([128, 2, EO // 2, 256], F8,
                                             tag="w1t")
                            nc.scalar.dma_start(w1t[:],
                                                w1_d.ap()[:, fo0:fo0 + 2])
                            for f2i in range(2):
                                fo = fo0 + f2i
                                f1_ps = ps_f1.tile([128, TSL], F32, tag="f1")
                                for q4 in range(EO // 2):
                                    nc.tensor.matmul(
                                        f1_ps[:, 0:t1 - t0],
                                        w1t[:, f2i, q4].rearrange(
                                            "p (two m) -> p two m", two=2),
                                        x1T[:, 2 * q4:2 * q4 + 2, t0:t1],
                                        start=q4 == 0,
                                        stop=q4 == EO // 2 - 1,
                                        perf_mode=DRSI)
                                nc.scalar.activation(hT[:, fo, t0:t1],
                                                     f1_ps[:, 0:t1 - t0],
                                                     AF.Relu,
                                                     bias=b1_sb[:, fo:fo + 1])

                    # wo/LN1 for the batch-0 half, then bridge the a2a1 +
                    # gather latency with ffn1 work on the finished half
                    FSPLIT = 26
                    wo_half(0)
                    with nc.named_scope("ffn1"), \
                         tc.tile_pool(name="ps_f1", bufs=3,
                                      space="PSUM") as ps_f1:
                        for m in range(2):
                            nc.vector.tensor_tensor(
                                x1b[:, m, :], x1n[:, m, :], b2_bc[:], OP.add)
                        ffn1_range(range(FSPLIT), 0, ps_f1)
                        wo_half(1)
                        for m in range(2, 4):
                            nc.vector.tensor_tensor(
                                x1b[:, m, :], x1n[:, m, :], b2_bc[:], OP.add)
                        ffn1_range(range(FSPLIT), 1, ps_f1)
                        ffn1_range(range(FSPLIT, FO), None, ps_f1)

                # ffn2 in two m-group passes (W2 streamed twice) so the
                # LN2/store epilogue of pass 0 overlaps pass 1's matmuls
                with nc.named_scope("ffn2_ln2"), \
                     tc.tile_pool(name="ps_f2", bufs=8, space="PSUM") as ps_f2:
                    w2r = w2_d.ap().rearrange("(fo p) e -> p fo e", p=128)
                    for g in range(2):
                        f2_ps = [ps_f2.tile([128, 512], F32, tag="f2",
                                            name=f"f2_{g}_{k}")
                                 for k in range(4)]
                        for fp in range(FO // 2):  # paired w2 loads
                            w2t = bh_w2.tile([128, 2, E], BF, tag="w2t")
                            eng = nc.scalar if fp % 2 == 0 else nc.sync
                            eng.dma_start(w2t[:], w2r[:, 2 * fp:2 * fp + 2, :])
                            for f2i in range(2):
                                fo = 2 * fp + f2i
                                for mi in range(2):
                                    m = 2 * g + mi
                                    for n in range(2):
                                        nc.tensor.matmul(
                                            f2_ps[2 * mi + n][:],
                                            hT[:, fo, 128 * m:128 * (m + 1)],
                                            w2t[:, f2i, 512 * n:512 * (n + 1)],
                                            start=fo == 0, stop=fo == FO - 1)
                        for mi in range(2):
                            m = 2 * g + mi
                            for n in range(2):
                                sl = slice(512 * n, 512 * (n + 1))
                                nc.vector.tensor_tensor(
                                    x1b[:, m, sl], f2_ps[2 * mi + n][:],
                                    x1b[:, m, sl], OP.add)
                            layernorm(x1b[:, m, :], g2_bc, be2_bc)
                            nc.sync.dma_start(out_dst[:, m, :], x1b[:, m, :])

    nc.compile()
    return nc


def _make_in_maps(inputs):
    x = np.asarray(inputs["x"], dtype=np.float32)
    Wq = np.asarray(inputs["Wq"], dtype=np.float32)
    Wk = np.asarray(inputs["Wk"], dtype=np.float32)
    Wv = np.asarray(inputs["Wv"], dtype=np.float32)
    Wo = np.asarray(inputs["Wo"], dtype=np.float32)

    xflat = x.reshape(NTOK, E)
    # [p, t-chunk, eo, t'] — one contiguous 8KB run per partition per chunk
    xT = np.ascontiguousarray(
        xflat.reshape(NTOK // 512, 512, EO_, 128).transpose(3, 0, 2, 1)
    ).astype(BF16)
    wo = np.ascontiguousarray(
        Wo.reshape(EO_, 128, E).transpose(1, 0, 2)).astype(BF16)
    w1b = np.ascontiguousarray(
        np.asarray(inputs["W1"], dtype=np.float32)
        .reshape(EO_, 128, 4 * E // 128, 128).transpose(1, 2, 0, 3)
    ).astype(F8E4)  # [p, fo, eo, hid]
    # interleave eo-pairs and reverse the hid columns for SwInterleave:
    # flat[2j + i] = w1b[p, fo, 2*q4 + i, 127 - j]
    w1 = np.ascontiguousarray(
        w1b.reshape(128, 4 * E // 128, EO_ // 2, 2, 128)[..., ::-1]
        .transpose(0, 1, 2, 4, 3)
        .reshape(128, 4 * E // 128, EO_ // 2, 256))
    w2 = np.asarray(inputs["W2"], dtype=np.float32).astype(BF16)
    b1s = np.ascontiguousarray(
        np.asarray(inputs["b1"], dtype=np.float32).reshape(4 * E // 128, 128).T)

    # mask patterns for the 4 diagonal-straddling [s=128, t=512] tiles,
    # pre-tiled to [p, q, t]
    masks4 = np.zeros((4, 128, 512), dtype=np.float32)
    srow = np.arange(128)[:, None]
    tcol = np.arange(512)[None, :]
    for q_ in range(4):
        masks4[q_] = (srow <= tcol - 128 * q_)
    masks = np.ascontiguousarray(masks4.transpose(1, 0, 2)).astype(BF16)

    ident = np.eye(128, dtype=np.float32)

    common = {
        "xT": xT,
        "wo": wo,
        "w1": w1,
        "w2": w2,
        "b1s": b1s,
        "bo": np.asarray(inputs["bo"], dtype=np.float32),
        "b1": np.asarray(inputs["b1"], dtype=np.float32),
        "b2": np.asarray(inputs["b2"], dtype=np.float32),
        "g1": np.asarray(inputs["g1"], dtype=np.float32),
        "be1": np.asarray(inputs["be1"], dtype=np.float32),
        "g2": np.asarray(inputs["g2"], dtype=np.float32),
        "be2": np.asarray(inputs["be2"], dtype=np.float32),
        "masks": masks,
        "id_bf": ident.astype(BF16),
    }
    in_maps = []
    for c in range(N_CORES):
        m = dict(common)
        def tile_w(W):
            # [E, 128] head-pair concat -> [p, eo, d]
            wc = np.concatenate([W[2 * c], W[2 * c + 1]], axis=1)
            return np.ascontiguousarray(
                wc.reshape(EO_, 128, 128).transpose(1, 0, 2)).astype(BF16)
        m["wq"] = tile_w(Wq)
        m["wk"] = tile_w(Wk)
        m["wv"] = tile_w(Wv)
        # per-core token rows: batch-0 [256c, 256c+256) then batch-1 same,
        # pre-tiled to [p, m, e]
        rows = np.concatenate([
            xflat[HSL * c:HSL * (c + 1)],
            xflat[T + HSL * c:T + HSL * (c + 1)]], axis=0)
        m["x_slice"] = np.ascontiguousarray(
            rows.reshape(TSL // 128, 128, E).transpose(1, 0, 2))
        in_maps.append(m)
    return in_maps


def _enable_trace_hook():
    """Register the axon NTFF profile hook (missing antenv.axon_hooks shim)."""
    import types
    import antenv  # noqa: F401

    if "antenv.axon_hooks" not in sys.modules:
        mod = types.ModuleType("antenv.axon_hooks")
        _hook = [None]
        mod.set_axon_ntff_profile_hook = lambda h: _hook.__setitem__(0, h)
        mod.get_axon_ntff_profile_hook = lambda: _hook[0]
        sys.modules["antenv.axon_hooks"] = mod
        antenv.axon_hooks = mod
    mod = sys.modules["antenv.axon_hooks"]
    if mod.get_axon_ntff_profile_hook() is None:
        from trn_agent_boot.trn_boot import _ntff_profile_via_ctypes
        mod.set_axon_ntff_profile_hook(
            _ntff_profile_via_ctypes("/opt/axon/libaxon_pjrt.so"))


def run(inputs, trace=False):
    """Returns (full_output [B,T,E] f32, BassKernelResults)."""
    from concourse import bass_utils

    if "nc" not in _cache:
        _cache["nc"] = _build()
    nc = _cache["nc"]
    in_maps = _make_in_maps(inputs)
    if trace:
        _enable_trace_hook()
    res = bass_utils.run_bass_kernel_spmd(
        nc, in_maps, core_ids=list(range(N_CORES)), trace=trace)
    full = np.empty((NTOK, E), dtype=np.float32)
    for c in range(N_CORES):
        o = res.results[c]["out"]
        full[HSL * c:HSL * (c + 1)] = o[:HSL]
        full[T + HSL * c:T + HSL * (c + 1)] = o[HSL:]
    return full.reshape(B, T, E), res


def kernel(**inputs):
    out, _ = run(inputs, trace=False)
    return out

